# revision 16
# baseline (speedup 1.0000x reference)
"""Trainium2 Bass kernel for a Tacotron-style encoder:
   embedding -> 3x (conv1d k=5 SAME + BN + ReLU) -> bidirectional LSTM (zoneout, eval).

Contract: kernel(**inputs) takes FULL unsharded inputs (as numpy arrays) and
returns the FULL [B, T, 2H] float32 output. Internally shards batch across 8
NeuronCores (data-parallel), runs a Bass/Tile kernel per core, and gathers.

Self-contained: hardcodes all shapes; does not read sibling files.
"""

import numpy as np

import concourse.bacc as bacc
import concourse.bass as bass
import concourse.tile as tile
from concourse import mybir
from concourse.bass_utils import run_bass_kernel_spmd

# Model dims (hardcoded from the problem spec)
B, T, V, E, H, F, K = 32, 512, 256, 512, 256, 512, 5
ZONEOUT = 0.1
BN_EPS = 1e-3
N_CORES = 8
B_CORE = B // N_CORES  # 4

F32 = mybir.dt.float32
F32R = mybir.dt.float32r
F16 = mybir.dt.float16
I32 = mybir.dt.int32

# Gate chunk permutation: Keras order (i, f, g, o) -> device order (i, f, o, g)
# so sigmoid covers chunks 0..5 and tanh covers chunks 6..7 contiguously.
_GATE_PERM = np.r_[0:2 * H, 3 * H:4 * H, 2 * H:3 * H]


def _r(x):
    """fp32r view of an SBUF AP holding fp32 data."""
    return x.bitcast(F32R)


def build_program(Tn=T, b_core=B_CORE, warm=48):
    """Build the per-core Bass program. Returns the Bacc object."""
    nc = bacc.Bacc(trn_type="TRN2", debug=False, num_devices=N_CORES)

    n_core = b_core * Tn  # tokens per core
    EC = E // 128   # 4 embedding-dim chunks
    FC = F // 128   # 4 feature chunks
    VC = V // 128   # 2 vocab chunks
    GC = 4 * H // 128  # 8 gate chunks
    HC = H // 128   # 2 hidden chunks

    # ---- DRAM I/O (per core) ----
    tok_d = nc.dram_tensor("tokens", [n_core], F32, kind="ExternalInput")
    viota_d = nc.dram_tensor("viota", [128, VC], F32, kind="ExternalInput")
    embw_d = nc.dram_tensor("embw", [128, VC, EC, 128], F32R, kind="ExternalInput")
    convw_d = nc.dram_tensor("convw", [3, FC, 128, FC, K, 128], F32R, kind="ExternalInput")
    cbias_d = nc.dram_tensor("cbias", [128, 3 * FC], F32, kind="ExternalInput")
    wx_d = nc.dram_tensor("wx", [128, 2, FC, GC, 128], F32R, kind="ExternalInput")
    wh_d = nc.dram_tensor("wh", [128, 2, HC, GC, 128], F16, kind="ExternalInput")
    lbias_d = nc.dram_tensor("lbias", [128, 2 * GC], F32, kind="ExternalInput")
    hout_d = nc.dram_tensor("hout", [2, 128, HC, Tn, b_core], F32, kind="ExternalOutput")

    with tile.TileContext(nc) as tc:
        with tc.tile_pool(name="const", bufs=1) as const, \
             tc.tile_pool(name="lstmw", bufs=1) as lstmw, \
             tc.tile_pool(name="xwp", bufs=1) as xwp, \
             tc.tile_pool(name="hbuf", bufs=1) as hbuf, \
             tc.tile_pool(name="xp", bufs=2) as xp:

            cb = const.tile([128, 3 * FC], F32)
            nc.sync.dma_start(out=cb[:], in_=cbias_d.ap())
            lb = const.tile([128, 2 * GC], F32)
            nc.sync.dma_start(out=lb[:], in_=lbias_d.ap())
            wh_sb = lstmw.tile([128, 2, HC, GC, 128], F16)
            nc.sync.dma_start(out=wh_sb[:], in_=wh_d.ap())

            viota = const.tile([128, VC], F32)
            nc.sync.dma_start(out=viota[:], in_=viota_d.ap())

            def fresh_x():
                xt = xp.tile([128, FC, b_core, Tn + 4], F32R, tag="x")
                nc.vector.memset(xt[:, :, :, 0:2].bitcast(F32), 0.0)
                nc.vector.memset(xt[:, :, :, Tn + 2:Tn + 4].bitcast(F32), 0.0)
                return xt

            # ---- embedding via one-hot matmul ----
            psb_cm = tc.tile_pool(name="psb", bufs=4, space="PSUM")
            psb = psb_cm.__enter__()
            with tc.tile_pool(name="embp", bufs=1) as embp:
                embw = embp.tile([128, VC, EC, 128], F32R)
                nc.sync.dma_start(out=embw[:], in_=embw_d.ap())

                tokb = embp.tile([128, n_core], F32)
                tok_ap = tok_d.ap()
                nc.sync.dma_start(
                    out=tokb[:],
                    in_=bass.AP(tensor=tok_ap.tensor, offset=0,
                                ap=[[0, 128]] + list(tok_ap.ap)),
                )
                oh = embp.tile([128, VC, n_core], F32R)
                for vc in range(VC):
                    nc.vector.tensor_scalar(
                        out=oh[:, vc, :], in0=tokb[:], scalar1=viota[:, vc:vc + 1],
                        scalar2=None, op0=mybir.AluOpType.is_equal,
                    )

                x0 = fresh_x()
                for mc in range(EC):
                    for b in range(b_core):
                        ps = psb.tile([128, Tn], F32, tag="ps")
                        for vc in range(VC):
                            nc.tensor.matmul(
                                out=ps[:],
                                lhsT=_r(embw[:, vc, mc, :]),
                                rhs=_r(oh[:, vc, b * Tn:(b + 1) * Tn]),
                                start=(vc == 0), stop=(vc == VC - 1),
                            )
                        nc.scalar.activation(
                            out=x0[:, mc, b, 2:Tn + 2], in_=ps[:],
                            func=mybir.ActivationFunctionType.Copy,
                        )

            # ---- 3 conv layers (BN folded; ReLU+bias fused on eviction) ----
            xcur = x0
            with tc.tile_pool(name="cwp", bufs=3) as cwp:
                for l in range(3):
                    xn = fresh_x()
                    for mc in range(FC):
                        wl = cwp.tile([128, FC, K, 128], F32R, tag="wl")
                        nc.sync.dma_start(out=wl[:], in_=convw_d.ap()[l][mc])
                        for b in range(b_core):
                            ps = psb.tile([128, Tn], F32, tag="ps")
                            nmm = FC * K
                            i = 0
                            for kc in range(FC):
                                for k in range(K):
                                    nc.tensor.matmul(
                                        out=ps[:],
                                        lhsT=_r(wl[:, kc, k, :]),
                                        rhs=_r(xcur[:, kc, b, k:k + Tn]),
                                        start=(i == 0), stop=(i == nmm - 1),
                                    )
                                    i += 1
                            nc.scalar.activation(
                                out=xn[:, mc, b, 2:Tn + 2], in_=ps[:],
                                func=mybir.ActivationFunctionType.Relu,
                                bias=cb[:, l * FC + mc:l * FC + mc + 1],
                            )
                    xcur = xn

            # ---- LSTM input projections xw = x @ Wx + b -> DRAM staging ----
            with tc.tile_pool(name="wxp", bufs=1) as wxp:
                wx_sb = wxp.tile([128, 2, FC, GC, 128], F32R)
                nc.sync.dma_start(out=wx_sb[:], in_=wx_d.ap())
                xw = []
                for d in range(2):
                    xwd = xwp.tile([128, GC, Tn, b_core], F16, tag=f"xw{d}",
                                   name=f"xw{d}")
                    for mc in range(GC):
                        for b in range(b_core):
                            ps = psb.tile([128, Tn], F32, tag="ps")
                            for kc in range(FC):
                                nc.tensor.matmul(
                                    out=ps[:],
                                    lhsT=_r(wx_sb[:, d, kc, mc, :]),
                                    rhs=_r(xcur[:, kc, b, 2:Tn + 2]),
                                    start=(kc == 0), stop=(kc == FC - 1),
                                )
                            nc.scalar.activation(
                                out=xwd[:, mc, :, b], in_=ps[:],
                                func=mybir.ActivationFunctionType.Identity,
                                bias=lb[:, d * GC + mc:d * GC + mc + 1],
                            )
                    xw.append(xwd)

            psb_cm.__exit__(None, None, None)

            # ---- recurrence ----
            h_sb = [hbuf.tile([128, HC, Tn, b_core], F32, tag=f"h{d}", name=f"h{d}")
                    for d in range(2)]

            WARM = warm if Tn >= 256 else 0
            SEG = 2 if Tn >= 256 else 1
            seg_len = Tn // SEG
            with tc.tile_pool(name="state", bufs=12) as stp, \
                 tc.tile_pool(name="ew", bufs=8) as ew, \
                 tc.tile_pool(name="psg", bufs=8, space="PSUM") as psg:

                sig = mybir.ActivationFunctionType.Sigmoid
                tanh = mybir.ActivationFunctionType.Tanh
                mult = mybir.AluOpType.mult
                add = mybir.AluOpType.add
                sub = mybir.AluOpType.subtract

                # chains: (d, seg). seg s covers proc positions
                # [s*seg_len - WARM*(s>0), (s+1)*seg_len); warmup steps emit
                # no output. Later segments start at slot 0; segment 0 is
                # staggered by WARM//2 so chains finish closer together.
                chains = []
                for d in range(2):
                    for s in range(SEG):
                        warm = WARM if s > 0 else 0
                        start = s * seg_len - warm
                        nsteps = seg_len + warm
                        delay = (WARM // 2) if (SEG > 1 and s == 0) else 0
                        c0 = stp.tile([128, HC, b_core], F32, tag="C", name="C0")
                        nc.vector.memset(c0[:], 0.0)
                        h0 = stp.tile([128, HC, b_core], F16, tag="Hst", name="H0")
                        nc.vector.memset(h0[:], 0.0)
                        chains.append({"d": d, "start": start, "warm": warm,
                                       "nsteps": nsteps, "delay": delay,
                                       "C": c0, "H": h0})

                nslots = max(c["delay"] + c["nsteps"] for c in chains)
                for k in range(nslots):
                    act = []
                    for ch in chains:
                        j = k - ch["delay"]
                        if j < 0 or j >= ch["nsteps"]:
                            continue
                        d = ch["d"]
                        p = ch["start"] + j
                        tt = p if d == 0 else Tn - 1 - p
                        st = {"ch": ch, "d": d, "tt": tt,
                              "out": j >= ch["warm"]}
                        act.append(st)

                    for st in act:
                        ps = psg.tile([128, GC, b_core], F32, tag="psg")
                        for mc in range(GC):
                            for kc in range(HC):
                                nc.tensor.matmul(
                                    out=ps[:, mc, :],
                                    lhsT=wh_sb[:, st["d"], kc, mc, :],
                                    rhs=st["ch"]["H"][:, kc, :],
                                    start=(kc == 0), stop=(kc == HC - 1),
                                )
                        st["ps"] = ps
                    for st in act:
                        gsb = ew.tile([128, GC, b_core], F32, tag="gsb")
                        nc.vector.tensor_tensor(
                            out=gsb[:], in0=st["ps"][:],
                            in1=xw[st["d"]][:, :, st["tt"], :], op=add)
                        st["gsb"] = gsb
                    for st in act:
                        S = ew.tile([128, GC, b_core], F32, tag="S")
                        nc.scalar.activation(out=S[:], in_=st["gsb"][:], func=sig)
                        st["S"] = S
                    for st in act:
                        m2 = ew.tile([128, HC, b_core], F32, tag="m2")
                        nc.gpsimd.tensor_tensor(
                            out=m2[:], in0=st["S"][:, 2:4, :],
                            in1=st["ch"]["C"][:], op=mult)
                        st["m2"] = m2
                    for st in act:
                        m1p = ew.tile([128, HC, b_core], F32, tag="m1p")
                        nc.vector.tensor_tensor(
                            out=m1p[:], in0=st["S"][:, 0:2, :],
                            in1=st["S"][:, 6:8, :], op=mult)
                        st["m1p"] = m1p
                    for st in act:
                        m1 = ew.tile([128, HC, b_core], F32, tag="m1")
                        nc.vector.scalar_tensor_tensor(
                            out=m1[:], in0=st["m1p"][:], scalar=2.0,
                            in1=st["S"][:, 0:2, :], op0=mult, op1=sub)
                        st["m1"] = m1
                    for st in act:
                        cn = ew.tile([128, HC, b_core], F32, tag="cn")
                        nc.vector.scalar_tensor_tensor(
                            out=cn[:], in0=st["m2"][:], scalar=1.0 - ZONEOUT,
                            in1=st["m1"][:], op0=mult, op1=add)
                        st["cn"] = cn
                    for st in act:
                        TC = ew.tile([128, HC, b_core], F32, tag="TC")
                        nc.scalar.activation(out=TC[:], in_=st["cn"][:], func=tanh)
                        st["TC"] = TC
                    for st in act:
                        Cn = stp.tile([128, HC, b_core], F32, tag="C", name="Cn")
                        nc.vector.scalar_tensor_tensor(
                            out=Cn[:], in0=st["ch"]["C"][:], scalar=ZONEOUT,
                            in1=st["cn"][:], op0=mult, op1=add)
                        st["ch"]["C"] = Cn
                    for st in act:
                        if st["out"]:
                            hview = h_sb[st["d"]][:, :, st["tt"], :]
                        else:
                            hw = ew.tile([128, HC, b_core], F32, tag="hw")
                            hview = hw[:]
                        nc.gpsimd.tensor_tensor(
                            out=hview, in0=st["S"][:, 4:6, :],
                            in1=st["TC"][:], op=mult)
                        st["hv"] = hview
                    for st in act:
                        Hn = stp.tile([128, HC, b_core], F16, tag="Hst", name="Hn")
                        nc.vector.scalar_tensor_tensor(
                            out=Hn[:], in0=st["ch"]["H"][:], scalar=ZONEOUT,
                            in1=st["hv"], op0=mult, op1=add)
                        st["ch"]["H"] = Hn

            for d in range(2):
                nc.sync.dma_start(out=hout_d.ap()[d], in_=h_sb[d][:])

    nc.compile()
    return nc


def prep_weights(emb, conv_w, conv_b, bn_gamma, bn_beta, bn_mean, bn_var,
                 lstm_wx, lstm_wh, lstm_b):
    """Host-side weight folding + layout. Returns dict of device arrays."""
    EC, FC, VC = E // 128, F // 128, V // 128
    GC, HC = 4 * H // 128, H // 128

    inv = bn_gamma / np.sqrt(bn_var + BN_EPS)              # [3, F]
    dev = {}
    dev["embw"] = np.ascontiguousarray(
        emb.reshape(VC, 128, EC, 128).transpose(1, 0, 2, 3)).astype(np.float32)

    cw = np.empty((3, FC, 128, FC, K, 128), np.float32)
    cbias = np.empty((128, 3 * FC), np.float32)
    for l in range(3):
        wf = conv_w[l] * inv[l][None, None, :]             # [K, F, F]
        cw[l] = wf.reshape(K, FC, 128, FC, 128).transpose(3, 2, 1, 0, 4)
        bf = (conv_b[l] - bn_mean[l]) * inv[l] + bn_beta[l]  # [F]
        cbias[:, l * FC:(l + 1) * FC] = bf.reshape(FC, 128).T
    dev["convw"] = cw
    dev["cbias"] = cbias

    wx = np.empty((128, 2, FC, GC, 128), np.float32)
    wh = np.empty((128, 2, HC, GC, 128), np.float16)
    lbias = np.empty((128, 2 * GC), np.float32)
    # g-gate columns (post-perm 3H:4H) carry an extra x2 so one sigmoid
    # computes all gates: tanh(g) = 2*sigmoid(2g) - 1.
    gsc = np.ones((4 * H,), np.float32)
    gsc[3 * H:] = 2.0
    for d in range(2):
        wxp = lstm_wx[d][:, _GATE_PERM] * gsc              # [F, 4H]
        wx[:, d] = wxp.reshape(FC, 128, GC, 128).transpose(1, 0, 2, 3)
        whp = (1.0 - ZONEOUT) * lstm_wh[d][:, _GATE_PERM] * gsc  # [H, 4H]
        wh[:, d] = whp.reshape(HC, 128, GC, 128).transpose(1, 0, 2, 3).astype(np.float16)
        lbias[:, d * GC:(d + 1) * GC] = (lstm_b[d][_GATE_PERM] * gsc).reshape(GC, 128).T
    dev["wx"] = wx
    dev["wh"] = wh
    dev["lbias"] = lbias
    dev["viota"] = np.arange(V, dtype=np.float32).reshape(VC, 128).T.copy()
    return dev


_CACHED_NC = None


def _get_nc():
    global _CACHED_NC
    if _CACHED_NC is None:
        _CACHED_NC = build_program()
    return _CACHED_NC


def run(inputs, trace=False, **spmd_kwargs):
    """Run on 8 cores. Returns (output [B, T, 2H] f32, BassKernelResults)."""
    nc = _get_nc()
    dev = prep_weights(
        inputs["emb"], inputs["conv_w"], inputs["conv_b"], inputs["bn_gamma"],
        inputs["bn_beta"], inputs["bn_mean"], inputs["bn_var"],
        inputs["lstm_wx"], inputs["lstm_wh"], inputs["lstm_b"])
    tokens = np.asarray(inputs["tokens"], np.int32)

    in_maps = []
    for i in range(N_CORES):
        m = dict(dev)
        m["tokens"] = np.ascontiguousarray(
            tokens[i * B_CORE:(i + 1) * B_CORE].reshape(-1).astype(np.float32))
        in_maps.append(m)

    res = run_bass_kernel_spmd(nc, in_maps, core_ids=list(range(N_CORES)),
                               trace=trace, **spmd_kwargs)

    out = np.empty((B, T, 2 * H), np.float32)
    for i in range(N_CORES):
        r = res.results[i]["hout"]            # [2, 128, HC, T, b_core]
        # h[d, t, b, hc*128 + p] = r[d, p, hc, t, b]
        h = r.transpose(0, 3, 4, 2, 1).reshape(2, T, B_CORE, 2 * H // 2)
        out[i * B_CORE:(i + 1) * B_CORE, :, 0:H] = h[0].transpose(1, 0, 2)
        out[i * B_CORE:(i + 1) * B_CORE, :, H:2 * H] = h[1].transpose(1, 0, 2)
    return out, res


def kernel(**inputs):
    return run(inputs, trace=False)[0]
